# revision 8
# baseline (speedup 1.0000x reference)
"""Chamfer distance loss kernel for Trainium2 (8 NeuronCores).

Problem: points1, points2 [8, 4096, 3] fp32 -> scalar loss.
Sharding: data-parallel over batch; core b handles batch b. Host averages the
8 per-batch losses.

Per-core algorithm:
  dist[i,j] = n1[i] + n2[j] - 2*x1[i].x2[j]  (squared L2)
  -dist computed directly on-chip:
    * TensorE: PSUM[i,j] = sum_k L[k,i]*R[k,j] where the K=21 rows are a
      3-level bf16 split of the coordinates (hi/lo/lo2) plus rows carrying
      -n_j/2 (3-level bf16 split), so PSUM = (x_i.x_j)_fp32ish - n_j/2.
    * ScalarE: D = Identity(PSUM*2 + (-n_i)) cast to fp16 => D = -dist.
    * VectorE: max-tree over the free dim (fp16, 2x mode) + reduce_max
      => -min_j dist[i,j] per point. Both directions are computed with the
      roles of the clouds swapped, so both reductions are free-dim reductions.
  Means: ones-vector matmul partition-sum of the per-point maxes, then scale
  by -1/4096.
"""

import numpy as np

N = 4096          # points per cloud
P = 128           # partitions
TT = N // P       # 32 column blocks
D3 = 3
JB = 512          # matmul moving free dim
HALF = N // 2     # per-PSUM-allocation j extent (4 banks)
B = 8             # batches / cores

_NC_CACHE = {}


def _build_nc():
    import concourse.bacc as bacc
    import concourse.tile as tile
    from concourse import mybir

    FP32 = mybir.dt.float32

    nc = bacc.Bacc("TRN2", target_bir_lowering=False, debug=False)
    p1 = nc.dram_tensor("points1", [N, D3], FP32, kind="ExternalInput").ap()
    p2 = nc.dram_tensor("points2", [N, D3], FP32, kind="ExternalInput").ap()
    out = nc.dram_tensor("loss", [1, 1], FP32, kind="ExternalOutput").ap()

    with tile.TileContext(nc) as tc:
        _emit(tc, p1, p2, out)

    nc.compile()
    return nc


def _emit(tc, p1, p2, out):
    import concourse.bass as bass  # noqa: F401
    from concourse import mybir
    from concourse.masks import make_identity

    FP32 = mybir.dt.float32
    BF16 = mybir.dt.bfloat16
    FP16 = mybir.dt.float16
    AX = mybir.AxisListType
    OP = mybir.AluOpType
    AF = mybir.ActivationFunctionType

    nc = tc.nc

    # Row spec: pairs of (L-side source, R-side source) per coordinate.
    # H = bf16 hi, L = lo, L2 = lo2 of the raw coordinate values.
    COORD_PAIRS = [
        ("H", "H"), ("H", "L"), ("H", "L2"), ("L", "H"), ("L", "L"), ("L2", "H"),
    ]
    NROWS = len(COORD_PAIRS) * D3 + 3  # + (ONE x norm-split) rows

    from contextlib import ExitStack
    with ExitStack() as ctx:
        consts = ctx.enter_context(tc.tile_pool(name="consts", bufs=1))

        ident = consts.tile([P, P], FP32, name="ident", tag="ident")
        make_identity(nc, ident)

        ones_col = consts.tile([P, 1], FP32, name="ones_col", tag="ones_col")
        nc.vector.memset(ones_col, 1.0)

        ones_blk = consts.tile([TT, P], BF16, name="ones_blk", tag="ones_blk")
        nc.vector.memset(ones_blk, 1.0)

        # Persistent per-direction operand buffers and biases.
        Lbufs, Rbufs, negNs = [], [], []
        for m in range(2):
            Lbufs.append(consts.tile([NROWS, N], BF16, name=f"Lbuf{m}", tag=f"Lbuf{m}"))
            Rbufs.append(consts.tile([NROWS, N], BF16, name=f"Rbuf{m}", tag=f"Rbuf{m}"))
            negNs.append(consts.tile([P, TT], FP32, name=f"negN{m}", tag=f"negN{m}"))
        RMAX = consts.tile([P, 2 * TT], FP32, name="RMAX", tag="RMAX")

        # ---------------- setup phase ----------------
        with tc.tile_pool(name="pst", bufs=2, space="PSUM") as pst, \
             tc.tile_pool(name="stmp", bufs=1) as stmp:
            for m, X in enumerate((p1, p2)):
                S = stmp.tile([P, TT, D3], FP32, name=f"S{m}", tag=f"S{m}")
                nc.sync.dma_start(out=S, in_=X.rearrange("(p t) d -> p t d", p=P))

                SQ = stmp.tile([P, TT, D3], FP32, name=f"SQ{m}", tag=f"SQ{m}")
                nc.vector.tensor_mul(SQ, S, S)
                NP_ = stmp.tile([P, TT], FP32, name=f"NP{m}", tag=f"NP{m}")
                nc.vector.tensor_reduce(out=NP_, in_=SQ, axis=AX.X, op=OP.add)
                # ACT bias for the direction where this cloud is the i-side.
                nc.vector.tensor_scalar_mul(negNs[m], NP_, -1.0)

                # Transpose coords: S [128, 96] -> TS [96, 128] (fp32), with
                # coordinate d landing in the contiguous partition block
                # [32*d, 32*d+32). One transpose per coordinate because the
                # stationary matmul operand allows only one free dim.
                TS = stmp.tile([TT * D3, P], FP32, name=f"TS{m}", tag=f"TS{m}")
                for dd in range(D3):
                    in_d = S[:, :, dd:dd + 1].rearrange("p t e -> p (t e)")
                    tps = pst.tile([TT, P], FP32, name=f"tps{m}_{dd}", tag="tps")
                    nc.tensor.transpose(tps, in_d, ident)
                    nc.scalar.copy(TS[dd * TT:(dd + 1) * TT, :], tps)

                # 3-level bf16 split of coords.
                H = stmp.tile([TT * D3, P], BF16, name=f"H{m}", tag=f"H{m}")
                nc.vector.tensor_copy(H, TS)
                r1 = stmp.tile([TT * D3, P], FP32, name=f"r1_{m}", tag=f"r1_{m}")
                nc.vector.tensor_sub(r1, TS, H)
                Lo = stmp.tile([TT * D3, P], BF16, name=f"Lo{m}", tag=f"Lo{m}")
                nc.vector.tensor_copy(Lo, r1)
                r2 = stmp.tile([TT * D3, P], FP32, name=f"r2_{m}", tag=f"r2_{m}")
                nc.vector.tensor_sub(r2, r1, Lo)
                Lo2 = stmp.tile([TT * D3, P], BF16, name=f"Lo2{m}", tag=f"Lo2{m}")
                nc.vector.tensor_copy(Lo2, r2)

                # Norms transposed: NP [128, 32] -> [32, 128], scaled by -1/2,
                # then 3-level bf16 split.
                tpn = pst.tile([TT, P], FP32, name=f"tpn{m}", tag="tpn")
                nc.tensor.transpose(tpn, NP_, ident)
                TNn = stmp.tile([TT, P], FP32, name=f"TNn{m}", tag=f"TNn{m}")
                nc.scalar.mul(TNn, tpn, -0.5)
                NH = stmp.tile([TT, P], BF16, name=f"NH{m}", tag=f"NH{m}")
                nc.vector.tensor_copy(NH, TNn)
                nr1 = stmp.tile([TT, P], FP32, name=f"nr1_{m}", tag=f"nr1_{m}")
                nc.vector.tensor_sub(nr1, TNn, NH)
                NL = stmp.tile([TT, P], BF16, name=f"NL{m}", tag=f"NL{m}")
                nc.vector.tensor_copy(NL, nr1)
                nr2 = stmp.tile([TT, P], FP32, name=f"nr2_{m}", tag=f"nr2_{m}")
                nc.vector.tensor_sub(nr2, nr1, NL)
                NL2 = stmp.tile([TT, P], BF16, name=f"NL2{m}", tag=f"NL2{m}")
                nc.vector.tensor_copy(NL2, nr2)

                coord_src = {"H": H, "L": Lo, "L2": Lo2}
                norm_rows = [NH, NL, NL2]

                # Assemble row buffers. Row r of Lbuf[m] / Rbuf[m] has
                # column c = 128*t + p <-> point p*32 + t.
                dma_engines = [nc.sync, nc.scalar]
                di = 0

                def row_dma(dst_buf, r, src2d):
                    nonlocal di
                    dst = dst_buf[r:r + 1, :].rearrange("a (t p) -> a t p", p=P)
                    dma_engines[di % len(dma_engines)].dma_start(out=dst, in_=src2d)
                    di += 1

                for ci in range(D3):
                    for pi, (lsrc, rsrc) in enumerate(COORD_PAIRS):
                        r = ci * len(COORD_PAIRS) + pi
                        lsrc2d = coord_src[lsrc][ci * TT:(ci + 1) * TT, :]
                        rsrc2d = coord_src[rsrc][ci * TT:(ci + 1) * TT, :]
                        row_dma(Lbufs[m], r, lsrc2d)
                        row_dma(Rbufs[m], r, rsrc2d)
                for k in range(3):
                    r = D3 * len(COORD_PAIRS) + k
                    row_dma(Lbufs[m], r, ones_blk)
                    row_dma(Rbufs[m], r, norm_rows[k])

        # ---------------- main loop ----------------
        with tc.tile_pool(name="psm", bufs=2, space="PSUM") as psm, \
             tc.tile_pool(name="dpool", bufs=2) as dpool, \
             tc.tile_pool(name="papool", bufs=2) as papool, \
             tc.tile_pool(name="pbpool", bufs=2) as pbpool:
            for d in range(2):
                Lb = Lbufs[0] if d == 0 else Lbufs[1]
                Rb = Rbufs[1] if d == 0 else Rbufs[0]
                bias = negNs[0] if d == 0 else negNs[1]
                for t in range(TT):
                    Dt = dpool.tile([P, N], FP16, name="Dt", tag="Dt")
                    for h in range(2):
                        ps = psm.tile([P, HALF], FP32, name="ps", tag="ps")
                        for u in range(HALF // JB):
                            j0 = h * HALF + u * JB
                            nc.tensor.matmul(
                                ps[:, u * JB:(u + 1) * JB],
                                lhsT=Lb[:, t * P:(t + 1) * P],
                                rhs=Rb[:, j0:j0 + JB],
                                start=True, stop=True,
                            )
                        nc.scalar.activation(
                            out=Dt[:, h * HALF:(h + 1) * HALF],
                            in_=ps,
                            func=AF.Identity,
                            bias=bias[:, t:t + 1],
                            scale=2.0,
                        )
                    PA = papool.tile([P, HALF], FP16, name="PA", tag="PA")
                    PB = pbpool.tile([P, HALF // 2], FP16, name="PB", tag="PB")
                    nc.vector.tensor_max(PA, Dt[:, :HALF], Dt[:, HALF:])
                    nc.vector.tensor_max(PB, PA[:, :1024], PA[:, 1024:2048])
                    nc.vector.tensor_max(PA[:, :512], PB[:, :512], PB[:, 512:1024])
                    nc.vector.tensor_max(PB[:, :256], PA[:, :256], PA[:, 256:512])
                    nc.vector.tensor_reduce(
                        out=RMAX[:, d * TT + t: d * TT + t + 1],
                        in_=PB[:, :256], axis=AX.X, op=OP.max,
                    )

        # ---------------- final reduction ----------------
        with tc.tile_pool(name="psf", bufs=1, space="PSUM") as psf, \
             tc.tile_pool(name="ftmp", bufs=1) as ftmp:
            pss = psf.tile([1, 2 * TT], FP32, name="pss")
            nc.tensor.matmul(pss, lhsT=ones_col, rhs=RMAX, start=True, stop=True)
            ssum = ftmp.tile([1, 1], FP32, name="ssum", tag="ssum")
            nc.vector.tensor_reduce(out=ssum, in_=pss, axis=AX.X, op=OP.add)
            res = ftmp.tile([1, 1], FP32, name="res", tag="res")
            nc.vector.tensor_scalar_mul(res, ssum, -1.0 / N)
            nc.sync.dma_start(out=out, in_=res)


def get_nc():
    if "nc" not in _NC_CACHE:
        _NC_CACHE["nc"] = _build_nc()
    return _NC_CACHE["nc"]


def kernel(points1, points2, **_ignored):
    from concourse.bass_utils import run_bass_kernel_spmd

    p1 = np.ascontiguousarray(np.asarray(points1, dtype=np.float32))
    p2 = np.ascontiguousarray(np.asarray(points2, dtype=np.float32))
    assert p1.shape == (B, N, D3) and p2.shape == (B, N, D3)

    nc = get_nc()
    in_maps = [
        {"points1": p1[b].reshape(N, D3), "points2": p2[b].reshape(N, D3)}
        for b in range(B)
    ]
    res = run_bass_kernel_spmd(nc, in_maps, core_ids=list(range(B)))
    losses = np.array(
        [res.results[b]["loss"][0, 0] for b in range(B)], dtype=np.float32
    )
    return np.float32(losses.mean())
